# revision 9
# baseline (speedup 1.0000x reference)
"""Trainium2 Bass kernel for AttnBlock (GroupNorm + single-head dense spatial
attention + NIN projections + residual), data-parallel over batch across 8
NeuronCores.

Layout strategy per core (2 images of [4096 tokens, 128 ch] each):
  - channels-on-partitions ("^T") layout for GN + NIN + logit matmuls (f32r)
  - S^T tiles [keys, queries] so softmax exp feeds P@V without transposing P
  - exp on ScalarE (PSUM -> SBUF fp16), denominator accumulated on VectorE
  - P@V accumulated over key chunks in PSUM; o-projection applied on the
    unnormalized output; 1/Z and residual applied on the final [q, d] tiles.
  - host folds bo + bv@Wo into the residual tensor (exact algebra).
"""

import numpy as np
from contextlib import ExitStack

import concourse.bass as bass
import concourse.bacc as bacc
import concourse.tile as tile
import concourse.mybir as mybir
from concourse import bass_utils

F32 = mybir.dt.float32
F32R = mybir.dt.float32r
F16 = mybir.dt.float16
BF16 = mybir.dt.bfloat16
FP8 = mybir.dt.float8e4

B, H, W, C = 16, 64, 64, 128
T = H * W                    # 4096 tokens per image
NCORES = 8
BPC = B // NCORES            # images per core
NT = T // 128                # 32 token tiles
QC = 1024                    # query chunk
NQC = T // QC                # 4 query chunks
NJ = NT                      # 32 key chunks
EPS = 1e-6
SCALE = float(C) ** -0.5
INV_N = 1.0 / (T * 4)        # group has 4 channels x T tokens

ts = bass.ts

_CACHE = {}


def _build(trace=False, debug=False):
    nc = bacc.Bacc("TRN2", target_bir_lowering=False, debug=False)

    x_d = nc.dram_tensor("x_sh", [BPC, T, C], F32, kind="ExternalInput")
    r_d = nc.dram_tensor("resid_sh", [BPC, T, C], F32, kind="ExternalInput")
    wq_d = nc.dram_tensor("wq", [C, C], F32, kind="ExternalInput")
    wk_d = nc.dram_tensor("wk", [C, C], F32, kind="ExternalInput")
    wv_d = nc.dram_tensor("wv", [C, C], F32, kind="ExternalInput")
    wo_d = nc.dram_tensor("wo", [C, C], F32, kind="ExternalInput")
    bq_d = nc.dram_tensor("bq", [C], F32, kind="ExternalInput")
    bk_d = nc.dram_tensor("bk", [C], F32, kind="ExternalInput")
    gns_d = nc.dram_tensor("gn_scale", [C], F32, kind="ExternalInput")
    gnb_d = nc.dram_tensor("gn_bias", [C], F32, kind="ExternalInput")
    idn_d = nc.dram_tensor("ident", [128, 128], F32, kind="ExternalInput")
    gm_d = nc.dram_tensor("gmat", [128, 128], F32, kind="ExternalInput")
    on_d = nc.dram_tensor("onescol", [128, 1], F16, kind="ExternalInput")
    out_d = nc.dram_tensor("out_sh", [BPC, T, C], F32, kind="ExternalOutput")
    dbg_z = dbg_zi = dbg_es = None
    if debug:
        dbg_z = nc.dram_tensor("dbg_z", [BPC, T], F32, kind="ExternalOutput")
        dbg_zi = nc.dram_tensor("dbg_zi", [BPC, NQC, 128, 8], F32, kind="ExternalOutput")
        dbg_es = nc.dram_tensor("dbg_es", [BPC, NQC, 128, QC], F32, kind="ExternalOutput")
        dbg_zt = nc.dram_tensor("dbg_zt", [BPC, NQC, 128, 8], F32, kind="ExternalOutput")

    with tile.TileContext(nc) as tc, ExitStack() as ctx:
        # ---- pools -------------------------------------------------------
        consts = ctx.enter_context(tc.tile_pool(name="consts", bufs=1))
        p_xn = ctx.enter_context(tc.tile_pool(name="xn", bufs=1))
        p_xt = ctx.enter_context(tc.tile_pool(name="xt", bufs=1))
        p_ht = ctx.enter_context(tc.tile_pool(name="ht", bufs=1))
        p_qt = ctx.enter_context(tc.tile_pool(name="qt", bufs=2))
        p_kt = ctx.enter_context(tc.tile_pool(name="kt", bufs=2))
        p_v = ctx.enter_context(tc.tile_pool(name="v", bufs=2))
        p_junk = ctx.enter_context(tc.tile_pool(name="junk", bufs=1))
        p_small = ctx.enter_context(tc.tile_pool(name="small", bufs=2))
        p_e = ctx.enter_context(tc.tile_pool(name="e", bufs=3))
        p_esum = ctx.enter_context(tc.tile_pool(name="esum", bufs=2))
        p_osb = ctx.enter_context(tc.tile_pool(name="osb", bufs=2))
        p_fin = ctx.enter_context(tc.tile_pool(name="fin", bufs=2))
        # PSUM pools
        p_ps_out = ctx.enter_context(tc.tile_pool(name="ps_out", bufs=1, space="PSUM"))
        p_ps_s = ctx.enter_context(tc.tile_pool(name="ps_s", bufs=2, space="PSUM"))
        p_ps_sc = ctx.enter_context(tc.tile_pool(name="ps_sc", bufs=2, space="PSUM"))

        # ---- constants ---------------------------------------------------
        wq_sb = consts.tile([C, C], F32R); nc.sync.dma_start(wq_sb[:], wq_d[:].bitcast(F32R))
        wk_sb = consts.tile([C, C], F32R); nc.sync.dma_start(wk_sb[:], wk_d[:].bitcast(F32R))
        wv_sb = consts.tile([C, C], F32R); nc.sync.dma_start(wv_sb[:], wv_d[:].bitcast(F32R))
        wo_sb = consts.tile([C, C], F32R); nc.sync.dma_start(wo_sb[:], wo_d[:].bitcast(F32R))
        idn_sb = consts.tile([128, 128], F32); nc.sync.dma_start(idn_sb[:], idn_d[:])
        gm_sb = consts.tile([128, 128], F32); nc.sync.dma_start(gm_sb[:], gm_d[:])
        on_sb = consts.tile([128, 1], F16); nc.sync.dma_start(on_sb[:], on_d[:])
        bq_sb = consts.tile([C, 1], F32)
        nc.sync.dma_start(bq_sb[:], bq_d.rearrange("(c one) -> c one", one=1))
        bk_sb = consts.tile([C, 1], F32)
        nc.sync.dma_start(bk_sb[:], bk_d.rearrange("(c one) -> c one", one=1))
        gns_sb = consts.tile([C, 1], F32)
        nc.sync.dma_start(gns_sb[:], gns_d.rearrange("(c one) -> c one", one=1))
        gnb_sb = consts.tile([C, 1], F32)
        nc.sync.dma_start(gnb_sb[:], gnb_d.rearrange("(c one) -> c one", one=1))

        eps_sb = consts.tile([128, 1], F32)
        nc.vector.memset(eps_sb[:], EPS)
        one1_sb = consts.tile([1, 1], F32)
        nc.vector.memset(one1_sb[:], 1.0)

        AL = mybir.AluOpType
        AF = mybir.ActivationFunctionType

        for b in range(BPC):
            # ================= phase A: load + transpose + groupnorm ======
            xn = p_xn.tile([128, T], F32, tag="xn")
            nc.sync.dma_start(xn[:].rearrange("p (t c) -> p t c", c=128),
                              x_d[b].rearrange("(t p) c -> p t c", p=128))
            xt = p_xt.tile([128, T], F32, tag="xt")
            for t in range(NT):
                tp = p_ps_sc.tile([128, 128], F32, tag="sc")
                nc.tensor.transpose(tp[:], xn[:, ts(t, 128)], idn_sb[:])
                if t % 2 == 0:
                    nc.vector.tensor_copy(xt[:, ts(t, 128)], tp[:])
                else:
                    nc.scalar.copy(xt[:, ts(t, 128)], tp[:])

            sv = p_small.tile([128, 16], F32, tag="sv")
            junk = p_junk.tile([128, T], FP8, tag="junk")
            nc.vector.tensor_reduce(sv[:, 0:1], xt[:], axis=mybir.AxisListType.X,
                                    op=AL.add)
            nc.scalar.activation(junk[:], xt[:], AF.Square, accum_out=sv[:, 1:2])
            gps = p_ps_sc.tile([128, 2], F32, tag="sc")
            nc.tensor.matmul(gps[:], gm_sb[:], sv[:, 0:2], start=True, stop=True)
            # [mean, meansq] -> a = rstd*gn_scale, bb = gn_bias - mean*a
            nc.vector.tensor_scalar_mul(sv[:, 2:4], gps[:], INV_N)
            nc.vector.tensor_mul(sv[:, 4:5], sv[:, 2:3], sv[:, 2:3])
            nc.vector.tensor_sub(sv[:, 5:6], sv[:, 3:4], sv[:, 4:5])
            nc.scalar.activation(sv[:, 6:7], sv[:, 5:6], AF.Sqrt, bias=eps_sb[:])
            nc.vector.reciprocal(sv[:, 7:8], sv[:, 6:7])
            nc.vector.tensor_mul(sv[:, 8:9], sv[:, 7:8], gns_sb[:])
            nc.vector.tensor_mul(sv[:, 9:10], sv[:, 2:3], sv[:, 8:9])
            nc.vector.tensor_sub(sv[:, 10:11], gnb_sb[:], sv[:, 9:10])
            ht = p_ht.tile([128, T], F32R, tag="ht")
            nc.vector.tensor_scalar(ht[:], xt[:], sv[:, 8:9], sv[:, 10:11],
                                    AL.mult, AL.add)

            # ================= phase B: q/k/v NINs ========================
            qt = p_qt.tile([128, T], F32R, tag="qt")
            kt = p_kt.tile([128, T], F32R, tag="kt")
            htr = ht[:]
            for n in range(T // 512):
                pq = p_ps_sc.tile([128, 512], F32, tag="sc")
                nc.tensor.matmul(pq[:], wq_sb[:],
                                 htr[:, ts(n, 512)], start=True, stop=True)
                nc.vector.tensor_scalar_add(qt[:, ts(n, 512)], pq[:], bq_sb[:])
                pk = p_ps_sc.tile([128, 512], F32, tag="sc")
                nc.tensor.matmul(pk[:], wk_sb[:],
                                 htr[:, ts(n, 512)], start=True, stop=True)
                nc.scalar.activation(kt[:, ts(n, 512)], pk[:], AF.Identity,
                                     bias=bk_sb[:])
            vt = p_v.tile([128, T], F16, tag="v")
            for t in range(NT):
                pv = p_ps_sc.tile([128, 128], F32, tag="sc")
                nc.tensor.matmul(pv[:], htr[:, ts(t, 128)],
                                 wv_sb[:], start=True, stop=True)
                if t % 2 == 0:
                    nc.vector.tensor_copy(vt[:, ts(t, 128)], pv[:])
                else:
                    nc.scalar.copy(vt[:, ts(t, 128)], pv[:])

            # ================= phase C: attention =========================
            qtr = qt[:]
            ktr = kt[:]
            for qc in range(NQC):
                outT = p_ps_out.tile([128, QC], F32, tag="outT")
                esum = p_esum.tile([128, QC], F16, tag="esum")
                for j in range(NJ):
                    sps = p_ps_s.tile([128, QC], F32, tag="s")
                    for hh in range(QC // 512):
                        nc.tensor.matmul(sps[:, ts(hh, 512)], ktr[:, ts(j, 128)],
                                         qtr[:, ts(2 * qc + hh, 512)],
                                         start=True, stop=True)
                    e = p_e.tile([128, QC], F16, tag="e")
                    nc.scalar.activation(e[:], sps[:], AF.Exp, scale=SCALE)
                    for hh in range(QC // 512):
                        nc.tensor.matmul(outT[:, ts(hh, 512)], vt[:, ts(j, 128)],
                                         e[:, ts(hh, 512)],
                                         start=(j == 0), stop=(j == NJ - 1))
                    if j == 0:
                        nc.vector.tensor_copy(esum[:], e[:])
                    else:
                        nc.vector.tensor_add(esum[:], esum[:], e[:])

                # ---- epilogue: Z, o-projection, normalize + residual -----
                zps = []
                for hh in range(QC // 512):
                    zp = p_ps_sc.tile([1, 512], F32, tag="sc")
                    nc.tensor.matmul(zp[:], on_sb[:], esum[:, ts(hh, 512)],
                                     start=True, stop=True)
                    zps.append(zp)
                z_sb = p_small.tile([1, QC], F32, tag="zsb")
                for hh, zp in enumerate(zps):
                    nc.scalar.copy(z_sb[:, ts(hh, 512)], zp[:])
                ztp = p_ps_sc.tile([128, 8], F32, tag="sc")
                for i8 in range(8):
                    nc.tensor.matmul(ztp[:, i8:i8 + 1], z_sb[0:1, ts(i8, 128)],
                                     one1_sb[:], start=True, stop=True)
                zinv = p_small.tile([128, 8], F32, tag="zinv")
                nc.vector.reciprocal(zinv[:], ztp[:])
                if debug:
                    nc.sync.dma_start(dbg_z[b][qc * QC:(qc + 1) * QC]
                                      .rearrange("(one q) -> one q", one=1), z_sb[:])
                    nc.sync.dma_start(dbg_zi[b][qc], zinv[:])
                    ztf = p_small.tile([128, 8], F32, tag="ztf")
                    nc.scalar.copy(ztf[:], ztp[:])
                    nc.sync.dma_start(dbg_zt[b][qc], ztf[:])
                    esf = p_osb.tile([128, QC], F32, tag="esf")
                    nc.vector.tensor_copy(esf[:], esum[:])
                    nc.sync.dma_start(dbg_es[b][qc], esf[:])

                outT_sb = p_osb.tile([128, QC], F32R, tag="osb")
                nc.scalar.copy(outT_sb[:], outT[:])
                projT_sb = p_osb.tile([128, QC], F32, tag="projsb")
                for hh in range(QC // 512):
                    pj = p_ps_sc.tile([128, 512], F32, tag="sc")
                    nc.tensor.matmul(pj[:], wo_sb[:],
                                     outT_sb[:, ts(hh, 512)],
                                     start=True, stop=True)
                    nc.vector.tensor_copy(projT_sb[:, ts(hh, 512)], pj[:])

                rn = p_fin.tile([128, QC], F32, tag="rn")
                nc.sync.dma_start(
                    rn[:].rearrange("p (t c) -> p t c", c=128),
                    r_d[b][qc * QC:(qc + 1) * QC, :]
                    .rearrange("(t p) c -> p t c", p=128))
                fin = p_fin.tile([128, QC], F32, tag="fin")
                for t8 in range(QC // 128):
                    pn = p_ps_sc.tile([128, 128], F32, tag="sc")
                    nc.tensor.transpose(pn[:], projT_sb[:, ts(t8, 128)], idn_sb[:])
                    nc.vector.scalar_tensor_tensor(
                        fin[:, ts(t8, 128)], pn[:], zinv[:, t8:t8 + 1],
                        rn[:, ts(t8, 128)], AL.mult, AL.add)
                nc.sync.dma_start(
                    out_d[b][qc * QC:(qc + 1) * QC, :]
                    .rearrange("(t p) c -> p t c", p=128),
                    fin[:].rearrange("p (t c) -> p t c", c=128))

    nc.compile()
    return nc


def _prep_core_inputs(inputs):
    x = np.ascontiguousarray(np.asarray(inputs["x"], dtype=np.float32))
    wq = np.asarray(inputs["wq"], np.float32)
    wk = np.asarray(inputs["wk"], np.float32)
    wv = np.asarray(inputs["wv"], np.float32)
    wo = np.asarray(inputs["wo"], np.float32)
    bq = np.asarray(inputs["bq"], np.float32)
    bk = np.asarray(inputs["bk"], np.float32)
    bv = np.asarray(inputs["bv"], np.float32)
    bo = np.asarray(inputs["bo"], np.float32)
    gns = np.asarray(inputs["gn_scale"], np.float32)
    gnb = np.asarray(inputs["gn_bias"], np.float32)

    xf = x.reshape(B, T, C)
    cvec = bo + bv @ wo          # exact fold of v-bias and o-bias
    resid = (xf + cvec[None, None, :]).astype(np.float32)

    ident = np.eye(128, dtype=np.float32)
    gmat = np.zeros((128, 128), np.float32)
    for g in range(32):
        gmat[g * 4:(g + 1) * 4, g * 4:(g + 1) * 4] = 1.0
    ones = np.ones((128, 1), np.float16)

    shared = dict(wq=wq, wk=wk, wv=wv, wo=wo, bq=bq, bk=bk,
                  gn_scale=gns, gn_bias=gnb,
                  ident=ident, gmat=gmat, onescol=ones)
    in_maps = []
    for c in range(NCORES):
        m = dict(shared)
        m["x_sh"] = np.ascontiguousarray(xf[c * BPC:(c + 1) * BPC])
        m["resid_sh"] = np.ascontiguousarray(resid[c * BPC:(c + 1) * BPC])
        in_maps.append(m)
    return in_maps


def kernel(**inputs):
    if "nc" not in _CACHE:
        _CACHE["nc"] = _build()
    nc = _CACHE["nc"]
    in_maps = _prep_core_inputs(inputs)
    res = bass_utils.run_bass_kernel_spmd(nc, in_maps, core_ids=list(range(NCORES)))
    outs = [res.results[c]["out_sh"] for c in range(NCORES)]
    full = np.concatenate(outs, axis=0).reshape(B, H, W, C)
    return full.astype(np.float32)
